# revision 41
# baseline (speedup 1.0000x reference)
"""Batch neighbor-list kernel for Trainium2 (Bass/Tile), 8 NeuronCores.

Problem: B=4 systems x N=2048 atoms, cubic box L=30 (cell read at runtime),
cutoff 5.0, min dist 0.01. For each system: pairwise minimum-image
difference vectors [N,N,3], distances [N,N], and mask [N,N], all zeroed
where the mask is False.

Strategy: circular half-pair coverage + PE-transpose mirror, one module
for all 8 cores.

The pair matrices are (anti)symmetric bitwise: fl(a-b) = -fl(b-a), the
wrap is odd, squares kill the sign; so dist/mask are exactly symmetric
and diff exactly antisymmetric. Working on the 16x16 grid of [128,128]
blocks per system, each core computes 8 row-strips (i-block m, j-blocks
m..m+8) — pairs at forward block-distance 0..8 — and mirrors the
k=1..7 blocks into (m+k, m) with a TensorE transpose (bit-exact for
f32); ScalarE copies PSUM->SBUF (scale=-1 for diff = exact negation;
Sign(dist^T) regenerates the mask byte). Distance-8 blocks are computed
by both of the two cores sharing a system (once per side), diagonal
blocks need no mirror. Core 2s+h handles system s with its atom blocks
rotated by 8h (host rolls outputs back), so strips m=0..7 cover blocks
8h..8h+7 and the two cores tile the full grid exactly once.

Strip pipeline (fused custom DVE ops; wrap(y) = y + ((y<-b)-(y>b))*2b is
the minimum image for |y| < 1.5L, valid since L > 2*cutoff):
  t    = sq(wrap(rowj_x - wpi_x))            NL_SQWRAP
  t   += sq(wrap(rowj_y - wpi_y))            NL_SQWRAP_ACC
  ssq  = t + sq(wrap(rowj_z - wpi_z))        NL_SQWRAP_ACC
  ssqm = (ssq < T_HI) * ssq                  scalar_tensor_tensor
  diff_c = wrap(rowj_c - wpi_c)*(ssq < T_HI) NL_WRAPMUL_LT (xyz interleaved)
  dist = Sqrt(ssqm); mask = Sign(dist) -> u8 ScalarE
T_HI/T_LO are exact f32 thresholds on dist^2 equivalent to the
reference's (sqrt > 0.01) & (sqrt < 5.0). Outputs use the T_HI cut plus
Sign(0)=0; pairs under T_LO are self-pairs (wrap diff exactly 0, dist
exactly 0), verified against the reference.
"""

import os
import sys

import numpy as np

if "/opt/trn_rl_repo" not in sys.path:
    sys.path.insert(0, "/opt/trn_rl_repo")

import concourse.bacc as bacc
import concourse.bass as bass
import concourse.mybir as mybir
from concourse import masks
from concourse.bass_utils import run_bass_kernel_spmd
from concourse.dve_ops import (
    _CUSTOM_DVE_ROW_BASE,
    _SUB_OPCODE_FOR_NAME,
    CUSTOM_DVE_SPECS,
    OPS,
    DveOp,
)
from concourse.dve_spec import C0, C1, C2, Spec, Src0, Src1, Zero, lower, sq
from concourse.dve_uop import DveOpSpec
from concourse.tile import TileContext
from concourse.tile_rust import add_dep_helper

B = 4
N = 2048
NCORES = 8
NB = N // 128  # 16 blocks per system
NSTRIP = 8  # strips per core
KMAX = 8  # forward block-distance per strip (9 blocks incl diagonal)
JL = (KMAX + 1) * 128  # 1152
EPS = 1e-7
CUTOFF = np.float32(5.0)
MIN_DIST = np.float32(0.01)
MIR_FLUSH = 4  # mirror blocks per DMA flush group

F32 = mybir.dt.float32
U8 = mybir.dt.uint8


def _mask_thresholds():
    """Exact f32 thresholds on dist^2 reproducing the reference's
    comparisons on dist = sqrt_rn(dist^2)."""

    def scan(center):
        c = np.float32(center)
        ups = [c]
        x = c
        for _ in range(512):
            x = np.nextafter(x, np.float32(np.inf))
            ups.append(x)
        x = c
        for _ in range(512):
            x = np.nextafter(x, np.float32(-np.inf))
            ups.append(x)
        return np.sort(np.array(ups, dtype=np.float32))

    a = scan(CUTOFF * CUTOFF)
    s = np.sqrt(a, dtype=np.float32)
    t_hi = a[s >= CUTOFF].min()
    b = scan(MIN_DIST * MIN_DIST)
    s = np.sqrt(b, dtype=np.float32)
    t_lo = b[s <= MIN_DIST].max()
    return np.float32(t_lo), np.float32(t_hi)


T_LO, T_HI = _mask_thresholds()


# ---- custom DVE ops ------------------------------------------------------ #


def _np_wrap(y, b):
    y = np.asarray(y, dtype=np.float32)
    two_b = np.float32(np.float32(b) + np.float32(b))
    g = (y < -np.float32(b)).astype(np.float32) - (y > np.float32(b)).astype(
        np.float32
    )
    return (y + two_b * g).astype(np.float32)


def _register(name, spec):
    for op in OPS:
        if op.name == name:
            return op
    from concourse.dve_spec import spec_leaves

    row = _CUSTOM_DVE_ROW_BASE + len(OPS)
    assert row < 0x20
    shas = {}
    for ver in ("v3", "v4"):
        s = DveOpSpec(
            name=name,
            opcode=row,
            uops=lower(spec, ver=ver),
            rd1_en=any(leaf is Src1 for leaf in spec_leaves(spec)),
        )
        shas[ver] = s.sha(ver)
    op = DveOp(name, spec, subdim=False, uops_sha=shas)
    OPS.append(op)
    CUSTOM_DVE_SPECS[name] = spec
    _SUB_OPCODE_FOR_NAME[name] = row
    return op


def _wrapped(y, b):
    return y + ((y < (Zero - b)) - (y > b)) * (b + b)


NL_SQWRAP = _register(
    "NL_SQWRAP",
    Spec(
        body=sq(_wrapped(Src0 + C0, C1)),
        reference=lambda in0, in1, s0, s1, imm2: _np_wrap(
            np.float32(in0) + np.float32(s0), s1
        )
        ** 2,
    ),
)
NL_SQWRAP_ACC = _register(
    "NL_SQWRAP_ACC",
    Spec(
        body=Src0 + sq(_wrapped(Src1 + C0, C1)),
        reference=lambda in0, in1, s0, s1, imm2: np.float32(in0)
        + _np_wrap(np.float32(in1) + np.float32(s0), s1) ** 2,
    ),
)
NL_WRAPMUL_LT = _register(
    "NL_WRAPMUL_LT",
    Spec(
        body=_wrapped(Src0 + C0, C1) * (Src1 < C2),
        reference=lambda in0, in1, s0, s1, imm2: _np_wrap(
            np.float32(in0) + np.float32(s0), s1
        )
        * (np.float32(in1) < np.float32(imm2)).astype(np.float32),
    ),
)


_COMPILED = {}
LAST_RESULT = None  # BassKernelResults of the most recent run (for profiling)


def _build_nc(L):
    """The per-core Bass program (identical on all 8 cores)."""
    nc = bacc.Bacc("TRN2", target_bir_lowering=False)
    halfL = float(np.float32(L) / np.float32(2.0))

    wpj = nc.declare_dram_parameter("wpj", [128, 3 * N], F32, isOutput=False)
    wpin = nc.declare_dram_parameter("wpin", [128, 3 * NSTRIP], F32,
                                     isOutput=False)
    # diff stored as 3 component planes [c, i, j]; host interleaves to
    # [i, j, c] while unsharding (keeps every DVE/ACT write and DMA run
    # contiguous on-device). Mirror blocks go to compact buffers (fully
    # contiguous DMA); the host scatters them into place.
    nmir = NSTRIP * (KMAX - 1)
    diff = nc.declare_dram_parameter("diff", [3 * N, N], F32, isOutput=True)
    dist = nc.declare_dram_parameter("dist", [N, N], F32, isOutput=True)
    mask = nc.declare_dram_parameter("mask", [N, N], U8, isOutput=True)
    diffm = nc.declare_dram_parameter("diffm", [nmir * 128, 384], F32,
                                      isOutput=True)
    distm_o = nc.declare_dram_parameter("distm", [nmir * 128, 128], F32,
                                        isOutput=True)
    maskm_o = nc.declare_dram_parameter("maskm", [nmir * 128, 128], U8,
                                        isOutput=True)

    ACT = mybir.ActivationFunctionType

    with TileContext(nc) as tc:
        with (
            tc.tile_pool(name="const", bufs=1) as cpool,
            tc.tile_pool(name="work", bufs=2) as pool,
            tc.tile_pool(name="outp", bufs=3) as opool,
            tc.tile_pool(name="mirr", bufs=3) as mpool,
            tc.tile_pool(name="psum", bufs=2, space="PSUM") as ppool,
        ):
            ident = cpool.tile([128, 128], F32)
            masks.make_identity(nc, ident[:, :])
            wpin_sb = cpool.tile([128, 3 * NSTRIP], F32)
            nc.sync.dma_start(out=wpin_sb[:, :], in_=wpin[:, :])
            # rowj broadcast from HBM; first chunk covers strip 0's j-range
            # [0, JL) so the DVE chain starts early. The rest is emitted
            # after strip 0's ops (Tile priority = program order) so the
            # first chunk wins the DMA queues.
            rowj = []
            for c in range(3):
                r = cpool.tile([128, N], F32, tag=f"rowj{c}")
                rowj.append(r)
            for c in range(3):
                nc.sync.dma_start(
                    out=rowj[c][:, 0:JL], in_=wpj[:, c * N : c * N + JL]
                )
            # single-row copy of the full positions + a ones column for the
            # PE-broadcast of the tail chunk (emitted after strip 0).
            wpj1 = cpool.tile([1, 3 * N], F32)
            nc.sync.dma_start(out=wpj1[:, :], in_=wpj[0:1, :])
            ones = cpool.tile([1, 128], F32)
            nc.vector.memset(ones[:, :], 1.0)

            for m in range(NSTRIP):
                j0 = m * 128
                sc = [wpin_sb[:, 3 * m + c : 3 * m + c + 1] for c in range(3)]
                js = slice(j0, j0 + JL)

                acc = pool.tile([128, JL], F32, tag="acc")
                nc.vector._custom_dve(
                    NL_SQWRAP, out=acc[:, :], in0=rowj[0][:, js],
                    s0=sc[0], s1=halfL,
                )
                acc2 = pool.tile([128, JL], F32, tag="acc2")
                nc.vector._custom_dve(
                    NL_SQWRAP_ACC, out=acc2[:, :], in0=acc[:, :],
                    in1=rowj[1][:, js], s0=sc[1], s1=halfL,
                )
                ssq = pool.tile([128, JL], F32, tag="ssq")
                nc.vector._custom_dve(
                    NL_SQWRAP_ACC, out=ssq[:, :], in0=acc2[:, :],
                    in1=rowj[2][:, js], s0=sc[2], s1=halfL,
                )
                ssqm = pool.tile([128, JL], F32, tag="ssqm")
                nc.vector.scalar_tensor_tensor(
                    out=ssqm[:, :], in0=ssq[:, :], scalar=float(T_HI),
                    in1=ssq[:, :],
                    op0=mybir.AluOpType.is_lt, op1=mybir.AluOpType.mult,
                )
                dstm = opool.tile([128, JL], F32, tag="dstm")
                nc.scalar.activation(out=dstm[:, :], in_=ssqm[:, :],
                                     func=ACT.Sqrt)
                dtile = opool.tile([128, 3 * JL], F32, tag="dtile")
                for c in range(3):
                    nc.vector._custom_dve(
                        NL_WRAPMUL_LT,
                        out=dtile[:, c * JL : (c + 1) * JL],
                        in0=rowj[c][:, js],
                        in1=ssq[:, :], s0=sc[c], s1=halfL, imm2=float(T_HI),
                    )
                m8 = opool.tile([128, JL], U8, tag="m8")
                nc.scalar.activation(out=m8[:, :], in_=dstm[:, :],
                                     func=ACT.Sign)

                rs = slice(j0, j0 + 128)
                for c in range(3):
                    nc.sync.dma_start(
                        out=diff[c * N + j0 : c * N + j0 + 128, js],
                        in_=dtile[:, c * JL : (c + 1) * JL],
                    )
                nc.sync.dma_start(out=dist[rs, js], in_=dstm[:, :])
                nc.sync.dma_start(out=mask[rs, js], in_=m8[:, :])

                if m == 0:
                    # second half of rowj via PE broadcast (ones^T @ row,
                    # exact 1.0*x) — saves 1.3MB of DMA; PE is idle here.
                    for c in range(3):
                        for lo, w in ((JL, 512), (JL + 512, N - JL - 512)):
                            bc = ppool.tile([128, w], F32, tag="pt4")
                            nc.tensor.matmul(
                                bc[:, :], ones[:, :],
                                wpj1[0:1, c * N + lo : c * N + lo + w],
                                start=True, stop=True,
                            )
                            nc.scalar.activation(
                                out=rowj[c][:, lo : lo + w], in_=bc[:, :],
                                func=ACT.Copy,
                            )

                # mirror blocks k=1..KMAX-1 into (m+k, m); k=KMAX is
                # computed directly by the sibling core.
                k = 1
                while k < KMAX:
                    kn = min(MIR_FLUSH, KMAX - k)
                    mdiff = mpool.tile([128, MIR_FLUSH * 384], F32, tag="mdiff")
                    mdist = mpool.tile([128, MIR_FLUSH * 128], F32, tag="mdist")
                    mmask = mpool.tile([128, MIR_FLUSH * 128], U8, tag="mmask")
                    # one PSUM tile per group: block g at bank g (512 f32)
                    pt4 = ppool.tile([128, kn * 512], F32, tag="pt4")
                    for g in range(kn):
                        fs = (k + g) * 128  # col offset within the strip
                        for c in range(3):
                            nc.tensor.transpose(
                                pt4[:, g * 512 + c * 128 : g * 512 + (c + 1) * 128],
                                dtile[:, c * JL + fs : c * JL + fs + 128],
                                ident[:, :],
                            )
                        nc.tensor.transpose(
                            pt4[:, g * 512 + 384 : g * 512 + 512],
                            dstm[:, fs : fs + 128], ident[:, :],
                        )
                    ptv = pt4[:, :].rearrange("p (g x) -> p g x", g=kn)
                    # diff mirrors: negate, batched; layout (g, c, j)
                    nc.scalar.activation(
                        out=mdiff[:, : kn * 384].rearrange(
                            "p (g x) -> p g x", g=kn
                        ),
                        in_=ptv[:, :, 0:384], func=ACT.Copy, scale=-1.0,
                    )
                    dsrc = ptv[:, :, 384:512]
                    nc.scalar.activation(
                        out=mdist[:, : kn * 128].rearrange(
                            "p (g j) -> p g j", g=kn
                        ),
                        in_=dsrc, func=ACT.Copy,
                    )
                    nc.scalar.activation(
                        out=mmask[:, : kn * 128].rearrange(
                            "p (g j) -> p g j", g=kn
                        ),
                        in_=dsrc, func=ACT.Sign,
                    )
                    ib = m * (KMAX - 1) + (k - 1)
                    nc.sync.dma_start(
                        out=diffm[ib * 128 : (ib + kn) * 128, :].rearrange(
                            "(g p) x -> p g x", p=128
                        ),
                        in_=mdiff[:, : kn * 384].rearrange(
                            "p (g x) -> p g x", g=kn
                        ),
                    )
                    nc.sync.dma_start(
                        out=distm_o[ib * 128 : (ib + kn) * 128, :].rearrange(
                            "(g p) x -> p g x", p=128
                        ),
                        in_=mdist[:, : kn * 128].rearrange(
                            "p (g x) -> p g x", g=kn
                        ),
                    )
                    nc.sync.dma_start(
                        out=maskm_o[ib * 128 : (ib + kn) * 128, :].rearrange(
                            "(g p) x -> p g x", p=128
                        ),
                        in_=mmask[:, : kn * 128].rearrange(
                            "p (g x) -> p g x", g=kn
                        ),
                    )
                    k += kn
    nc.compile()
    return nc


def _get_nc(L):
    key = float(L)
    if key not in _COMPILED:
        _COMPILED[key] = _build_nc(L)
    return _COMPILED[key]


def _wrap_positions_host(pos, cell, inv_cell):
    # replicate reference._wrap_positions in float32 numpy
    sp = (pos @ inv_cell + np.float32(EPS)).astype(np.float32)
    sp = np.mod(sp, np.float32(1.0))
    sp = (sp - np.float32(EPS)).astype(np.float32)
    return (sp @ cell).astype(np.float32)


def kernel(positions, cells, n_atoms):
    positions = np.asarray(positions, dtype=np.float32)
    cells = np.asarray(cells, dtype=np.float32)
    b = cells.shape[0] // 3
    n = positions.shape[0] // b
    assert b == B and n == N, (b, n)
    pos = positions.reshape(B, N, 3)
    cell = cells.reshape(B, 3, 3)

    # wrapped positions per batch (host; O(N) prep)
    wps = []
    L = None
    for i in range(B):
        inv_cell = np.linalg.inv(cell[i]).astype(np.float32)
        wps.append(_wrap_positions_host(pos[i], cell[i], inv_cell))
        Li = float(cell[i][0, 0])
        assert abs(cell[i][0, 0] - cell[i][1, 1]) < 1e-6
        assert abs(cell[i][0, 0] - cell[i][2, 2]) < 1e-6
        if L is None:
            L = Li
        else:
            assert abs(L - Li) < 1e-6

    nc = _get_nc(L)

    in_maps = []
    for core in range(NCORES):
        s = core // 2
        h = core % 2
        wp = np.roll(wps[s], -1024 * h, axis=0)  # rotated system view
        # replicated across partitions so the load DMA reads distinct
        # addresses at full HBM rate (a stride-0 broadcast read is
        # bank-serialized and ~3x slower).
        wpj = np.ascontiguousarray(
            np.broadcast_to(wp.T.reshape(1, 3 * N), (128, 3 * N))
        )
        wpin = np.empty((128, 3 * NSTRIP), dtype=np.float32)
        for m in range(NSTRIP):
            wpin[:, 3 * m : 3 * m + 3] = -wp[m * 128 : (m + 1) * 128, :]
        in_maps.append({"wpj": wpj, "wpin": wpin})

    res = run_bass_kernel_spmd(nc, in_maps, core_ids=list(range(NCORES)))
    global LAST_RESULT
    LAST_RESULT = res

    # assemble: roll the rotated cores' outputs back and merge (each true
    # block is written by exactly one core; unwritten regions are zero).
    # diff arrives as component planes [3, N, N] -> interleave to [N, N, 3].
    edge_diff = np.zeros((B, N, N, 3), dtype=np.float32)
    edge_dist = np.zeros((B, N, N), dtype=np.float32)
    edge_mask = np.zeros((B, N, N), dtype=np.uint8)
    for core in range(NCORES):
        s = core // 2
        h = core % 2
        out = res.results[core]
        od = out["diff"].reshape(3, N, N)
        osd = out["dist"]
        om = out["mask"]
        if h:
            od = np.roll(od, (1024, 1024), axis=(1, 2))
            osd = np.roll(osd, (1024, 1024), axis=(0, 1))
            om = np.roll(om, (1024, 1024), axis=(0, 1))
        edge_diff[s] += np.moveaxis(od, 0, -1)
        edge_dist[s] += osd
        edge_mask[s] |= om
        # scatter compact mirror blocks: block (m+k, m) in local slots
        dm = out["diffm"].reshape(NSTRIP, KMAX - 1, 128, 3, 128)
        sm = out["distm"].reshape(NSTRIP, KMAX - 1, 128, 128)
        mm = out["maskm"].reshape(NSTRIP, KMAX - 1, 128, 128)
        for m in range(NSTRIP):
            for k in range(1, KMAX):
                trb = ((m + k + 8 * h) % NB) * 128
                tcb = ((m + 8 * h) % NB) * 128
                edge_diff[s, trb : trb + 128, tcb : tcb + 128] = np.moveaxis(
                    dm[m, k - 1], 1, 2
                )
                edge_dist[s, trb : trb + 128, tcb : tcb + 128] = sm[m, k - 1]
                edge_mask[s, trb : trb + 128, tcb : tcb + 128] = mm[m, k - 1]
    return edge_diff, edge_dist, edge_mask.astype(bool)


# revision 43
# speedup vs baseline: 1.1333x; 1.1333x over previous
"""Batch neighbor-list kernel for Trainium2 (Bass/Tile), 8 NeuronCores.

Problem: B=4 systems x N=2048 atoms, cubic box L=30 (cell read at runtime),
cutoff 5.0, min dist 0.01. For each system: pairwise minimum-image
difference vectors [N,N,3], distances [N,N], and mask [N,N], all zeroed
where the mask is False.

Strategy: circular half-pair coverage + PE-transpose mirror, one module
for all 8 cores.

The pair matrices are (anti)symmetric bitwise: fl(a-b) = -fl(b-a), the
wrap is odd, squares kill the sign; so dist/mask are exactly symmetric
and diff exactly antisymmetric. Working on the 16x16 grid of [128,128]
blocks per system, each core computes 8 row-strips (i-block m, j-blocks
m..m+8) — pairs at forward block-distance 0..8 — and mirrors the
k=1..7 blocks into (m+k, m) with a TensorE transpose (bit-exact for
f32); ScalarE copies PSUM->SBUF (scale=-1 for diff = exact negation;
Sign(dist^T) regenerates the mask byte). Distance-8 blocks are computed
by both of the two cores sharing a system (once per side), diagonal
blocks need no mirror. Core 2s+h handles system s with its atom blocks
rotated by 8h (host rolls outputs back), so strips m=0..7 cover blocks
8h..8h+7 and the two cores tile the full grid exactly once.

Strip pipeline (fused custom DVE ops; wrap(y) = y + ((y<-b)-(y>b))*2b is
the minimum image for |y| < 1.5L, valid since L > 2*cutoff):
  t    = sq(wrap(rowj_x - wpi_x))            NL_SQWRAP
  t   += sq(wrap(rowj_y - wpi_y))            NL_SQWRAP_ACC
  ssq  = t + sq(wrap(rowj_z - wpi_z))        NL_SQWRAP_ACC
  ssqm = (ssq < T_HI) * ssq                  scalar_tensor_tensor
  diff_c = wrap(rowj_c - wpi_c)*(ssq < T_HI) NL_WRAPMUL_LT (xyz interleaved)
  dist = Sqrt(ssqm); mask = Sign(dist) -> u8 ScalarE
T_HI/T_LO are exact f32 thresholds on dist^2 equivalent to the
reference's (sqrt > 0.01) & (sqrt < 5.0). Outputs use the T_HI cut plus
Sign(0)=0; pairs under T_LO are self-pairs (wrap diff exactly 0, dist
exactly 0), verified against the reference.
"""

import os
import sys

import numpy as np

if "/opt/trn_rl_repo" not in sys.path:
    sys.path.insert(0, "/opt/trn_rl_repo")

import concourse.bacc as bacc
import concourse.bass as bass
import concourse.mybir as mybir
from concourse import masks
from concourse.bass_utils import run_bass_kernel_spmd
from concourse.dve_ops import (
    _CUSTOM_DVE_ROW_BASE,
    _SUB_OPCODE_FOR_NAME,
    CUSTOM_DVE_SPECS,
    OPS,
    DveOp,
)
from concourse.dve_spec import C0, C1, C2, Spec, Src0, Src1, Zero, lower, sq
from concourse.dve_uop import DveOpSpec
from concourse.tile import TileContext
from concourse.tile_rust import add_dep_helper

B = 4
N = 2048
NCORES = 8
NB = N // 128  # 16 blocks per system
NSTRIP = 8  # strips per core
KMAX = 8  # forward block-distance per strip (9 blocks incl diagonal)
JL = (KMAX + 1) * 128  # 1152
EPS = 1e-7
CUTOFF = np.float32(5.0)
MIN_DIST = np.float32(0.01)
MIR_FLUSH = 4  # mirror blocks per DMA flush group

F32 = mybir.dt.float32
U8 = mybir.dt.uint8


def _mask_thresholds():
    """Exact f32 thresholds on dist^2 reproducing the reference's
    comparisons on dist = sqrt_rn(dist^2)."""

    def scan(center):
        c = np.float32(center)
        ups = [c]
        x = c
        for _ in range(512):
            x = np.nextafter(x, np.float32(np.inf))
            ups.append(x)
        x = c
        for _ in range(512):
            x = np.nextafter(x, np.float32(-np.inf))
            ups.append(x)
        return np.sort(np.array(ups, dtype=np.float32))

    a = scan(CUTOFF * CUTOFF)
    s = np.sqrt(a, dtype=np.float32)
    t_hi = a[s >= CUTOFF].min()
    b = scan(MIN_DIST * MIN_DIST)
    s = np.sqrt(b, dtype=np.float32)
    t_lo = b[s <= MIN_DIST].max()
    return np.float32(t_lo), np.float32(t_hi)


T_LO, T_HI = _mask_thresholds()


# ---- custom DVE ops ------------------------------------------------------ #


def _np_wrap(y, b):
    y = np.asarray(y, dtype=np.float32)
    two_b = np.float32(np.float32(b) + np.float32(b))
    g = (y < -np.float32(b)).astype(np.float32) - (y > np.float32(b)).astype(
        np.float32
    )
    return (y + two_b * g).astype(np.float32)


def _register(name, spec):
    for op in OPS:
        if op.name == name:
            return op
    from concourse.dve_spec import spec_leaves

    row = _CUSTOM_DVE_ROW_BASE + len(OPS)
    assert row < 0x20
    shas = {}
    for ver in ("v3", "v4"):
        s = DveOpSpec(
            name=name,
            opcode=row,
            uops=lower(spec, ver=ver),
            rd1_en=any(leaf is Src1 for leaf in spec_leaves(spec)),
        )
        shas[ver] = s.sha(ver)
    op = DveOp(name, spec, subdim=False, uops_sha=shas)
    OPS.append(op)
    CUSTOM_DVE_SPECS[name] = spec
    _SUB_OPCODE_FOR_NAME[name] = row
    return op


def _wrapped(y, b):
    return y + ((y < (Zero - b)) - (y > b)) * (b + b)


NL_SQWRAP = _register(
    "NL_SQWRAP",
    Spec(
        body=sq(_wrapped(Src0 + C0, C1)),
        reference=lambda in0, in1, s0, s1, imm2: _np_wrap(
            np.float32(in0) + np.float32(s0), s1
        )
        ** 2,
    ),
)
NL_SQWRAP_ACC = _register(
    "NL_SQWRAP_ACC",
    Spec(
        body=Src0 + sq(_wrapped(Src1 + C0, C1)),
        reference=lambda in0, in1, s0, s1, imm2: np.float32(in0)
        + _np_wrap(np.float32(in1) + np.float32(s0), s1) ** 2,
    ),
)
NL_WRAPMUL_LT = _register(
    "NL_WRAPMUL_LT",
    Spec(
        body=_wrapped(Src0 + C0, C1) * (Src1 < C2),
        reference=lambda in0, in1, s0, s1, imm2: _np_wrap(
            np.float32(in0) + np.float32(s0), s1
        )
        * (np.float32(in1) < np.float32(imm2)).astype(np.float32),
    ),
)


_COMPILED = {}
LAST_RESULT = None  # BassKernelResults of the most recent run (for profiling)


def _build_nc(L):
    """The per-core Bass program (identical on all 8 cores)."""
    nc = bacc.Bacc("TRN2", target_bir_lowering=False)
    halfL = float(np.float32(L) / np.float32(2.0))

    wpj = nc.declare_dram_parameter("wpj", [128, 3 * N], F32, isOutput=False)
    wpin = nc.declare_dram_parameter("wpin", [128, 3 * NSTRIP], F32,
                                     isOutput=False)
    # diff stored as 3 component planes [c, i, j]; host interleaves to
    # [i, j, c] while unsharding (keeps every DVE/ACT write and DMA run
    # contiguous on-device). Mirror blocks go to compact buffers (fully
    # contiguous DMA); the host scatters them into place.
    nmir = NSTRIP * (KMAX - 1)
    diff = nc.declare_dram_parameter("diff", [3 * N, N], F32, isOutput=True)
    dist = nc.declare_dram_parameter("dist", [N, N], F32, isOutput=True)
    mask = nc.declare_dram_parameter("mask", [N, N], U8, isOutput=True)
    diffm = nc.declare_dram_parameter("diffm", [nmir * 128, 384], F32,
                                      isOutput=True)
    distm_o = nc.declare_dram_parameter("distm", [nmir * 128, 128], F32,
                                        isOutput=True)
    maskm_o = nc.declare_dram_parameter("maskm", [nmir * 128, 128], U8,
                                        isOutput=True)

    ACT = mybir.ActivationFunctionType

    with TileContext(nc) as tc:
        with (
            tc.tile_pool(name="const", bufs=1) as cpool,
            tc.tile_pool(name="work", bufs=2) as pool,
            tc.tile_pool(name="outp", bufs=3) as opool,
            tc.tile_pool(name="mirr", bufs=3) as mpool,
            tc.tile_pool(name="psum", bufs=2, space="PSUM") as ppool,
        ):
            ident = cpool.tile([128, 128], F32)
            masks.make_identity(nc, ident[:, :])
            wpin_sb = cpool.tile([128, 3 * NSTRIP], F32)
            nc.sync.dma_start(out=wpin_sb[:, :], in_=wpin[:, :])
            # rowj broadcast from HBM; first chunk covers strip 0's j-range
            # [0, JL) so the DVE chain starts early. The rest is emitted
            # after strip 0's ops (Tile priority = program order) so the
            # first chunk wins the DMA queues.
            rowj = []
            for c in range(3):
                r = cpool.tile([128, N], F32, tag=f"rowj{c}")
                rowj.append(r)
            for c in range(3):
                nc.sync.dma_start(
                    out=rowj[c][:, 0:JL], in_=wpj[:, c * N : c * N + JL]
                )

            for m in range(NSTRIP):
                j0 = m * 128
                sc = [wpin_sb[:, 3 * m + c : 3 * m + c + 1] for c in range(3)]
                js = slice(j0, j0 + JL)

                acc = pool.tile([128, JL], F32, tag="acc")
                nc.vector._custom_dve(
                    NL_SQWRAP, out=acc[:, :], in0=rowj[0][:, js],
                    s0=sc[0], s1=halfL,
                )
                acc2 = pool.tile([128, JL], F32, tag="acc2")
                nc.vector._custom_dve(
                    NL_SQWRAP_ACC, out=acc2[:, :], in0=acc[:, :],
                    in1=rowj[1][:, js], s0=sc[1], s1=halfL,
                )
                ssq = pool.tile([128, JL], F32, tag="ssq")
                nc.vector._custom_dve(
                    NL_SQWRAP_ACC, out=ssq[:, :], in0=acc2[:, :],
                    in1=rowj[2][:, js], s0=sc[2], s1=halfL,
                )
                ssqm = pool.tile([128, JL], F32, tag="ssqm")
                nc.vector.scalar_tensor_tensor(
                    out=ssqm[:, :], in0=ssq[:, :], scalar=float(T_HI),
                    in1=ssq[:, :],
                    op0=mybir.AluOpType.is_lt, op1=mybir.AluOpType.mult,
                )
                dstm = opool.tile([128, JL], F32, tag="dstm")
                nc.scalar.activation(out=dstm[:, :], in_=ssqm[:, :],
                                     func=ACT.Sqrt)
                dtile = opool.tile([128, 3 * JL], F32, tag="dtile")
                for c in range(3):
                    nc.vector._custom_dve(
                        NL_WRAPMUL_LT,
                        out=dtile[:, c * JL : (c + 1) * JL],
                        in0=rowj[c][:, js],
                        in1=ssq[:, :], s0=sc[c], s1=halfL, imm2=float(T_HI),
                    )
                m8 = opool.tile([128, JL], U8, tag="m8")
                nc.scalar.activation(out=m8[:, :], in_=dstm[:, :],
                                     func=ACT.Sign)

                rs = slice(j0, j0 + 128)
                for c in range(3):
                    nc.sync.dma_start(
                        out=diff[c * N + j0 : c * N + j0 + 128, js],
                        in_=dtile[:, c * JL : (c + 1) * JL],
                    )
                nc.sync.dma_start(out=dist[rs, js], in_=dstm[:, :])
                nc.sync.dma_start(out=mask[rs, js], in_=m8[:, :])

                if m == 0:
                    # second half of the rowj load, after strip 0's ops
                    for c in range(3):
                        nc.sync.dma_start(
                            out=rowj[c][:, JL:N],
                            in_=wpj[:, c * N + JL : (c + 1) * N],
                        )

                # mirror blocks k=1..KMAX-1 into (m+k, m); k=KMAX is
                # computed directly by the sibling core.
                k = 1
                while k < KMAX:
                    kn = min(MIR_FLUSH, KMAX - k)
                    mdiff = mpool.tile([128, MIR_FLUSH * 384], F32, tag="mdiff")
                    mdist = mpool.tile([128, MIR_FLUSH * 128], F32, tag="mdist")
                    mmask = mpool.tile([128, MIR_FLUSH * 128], U8, tag="mmask")
                    # one PSUM tile per group: block g at bank g (512 f32)
                    pt4 = ppool.tile([128, kn * 512], F32, tag="pt4")
                    for g in range(kn):
                        fs = (k + g) * 128  # col offset within the strip
                        for c in range(3):
                            nc.tensor.transpose(
                                pt4[:, g * 512 + c * 128 : g * 512 + (c + 1) * 128],
                                dtile[:, c * JL + fs : c * JL + fs + 128],
                                ident[:, :],
                            )
                        nc.tensor.transpose(
                            pt4[:, g * 512 + 384 : g * 512 + 512],
                            dstm[:, fs : fs + 128], ident[:, :],
                        )
                    ptv = pt4[:, :].rearrange("p (g x) -> p g x", g=kn)
                    # diff mirrors: negate, batched; layout (g, c, j)
                    nc.scalar.activation(
                        out=mdiff[:, : kn * 384].rearrange(
                            "p (g x) -> p g x", g=kn
                        ),
                        in_=ptv[:, :, 0:384], func=ACT.Copy, scale=-1.0,
                    )
                    dsrc = ptv[:, :, 384:512]
                    nc.scalar.activation(
                        out=mdist[:, : kn * 128].rearrange(
                            "p (g j) -> p g j", g=kn
                        ),
                        in_=dsrc, func=ACT.Copy,
                    )
                    nc.scalar.activation(
                        out=mmask[:, : kn * 128].rearrange(
                            "p (g j) -> p g j", g=kn
                        ),
                        in_=dsrc, func=ACT.Sign,
                    )
                    ib = m * (KMAX - 1) + (k - 1)
                    nc.sync.dma_start(
                        out=diffm[ib * 128 : (ib + kn) * 128, :].rearrange(
                            "(g p) x -> p g x", p=128
                        ),
                        in_=mdiff[:, : kn * 384].rearrange(
                            "p (g x) -> p g x", g=kn
                        ),
                    )
                    nc.sync.dma_start(
                        out=distm_o[ib * 128 : (ib + kn) * 128, :].rearrange(
                            "(g p) x -> p g x", p=128
                        ),
                        in_=mdist[:, : kn * 128].rearrange(
                            "p (g x) -> p g x", g=kn
                        ),
                    )
                    nc.sync.dma_start(
                        out=maskm_o[ib * 128 : (ib + kn) * 128, :].rearrange(
                            "(g p) x -> p g x", p=128
                        ),
                        in_=mmask[:, : kn * 128].rearrange(
                            "p (g x) -> p g x", g=kn
                        ),
                    )
                    k += kn
    nc.compile()
    return nc


def _get_nc(L):
    key = float(L)
    if key not in _COMPILED:
        _COMPILED[key] = _build_nc(L)
    return _COMPILED[key]


def _wrap_positions_host(pos, cell, inv_cell):
    # replicate reference._wrap_positions in float32 numpy
    sp = (pos @ inv_cell + np.float32(EPS)).astype(np.float32)
    sp = np.mod(sp, np.float32(1.0))
    sp = (sp - np.float32(EPS)).astype(np.float32)
    return (sp @ cell).astype(np.float32)


def kernel(positions, cells, n_atoms):
    positions = np.asarray(positions, dtype=np.float32)
    cells = np.asarray(cells, dtype=np.float32)
    b = cells.shape[0] // 3
    n = positions.shape[0] // b
    assert b == B and n == N, (b, n)
    pos = positions.reshape(B, N, 3)
    cell = cells.reshape(B, 3, 3)

    # wrapped positions per batch (host; O(N) prep)
    wps = []
    L = None
    for i in range(B):
        inv_cell = np.linalg.inv(cell[i]).astype(np.float32)
        wps.append(_wrap_positions_host(pos[i], cell[i], inv_cell))
        Li = float(cell[i][0, 0])
        assert abs(cell[i][0, 0] - cell[i][1, 1]) < 1e-6
        assert abs(cell[i][0, 0] - cell[i][2, 2]) < 1e-6
        if L is None:
            L = Li
        else:
            assert abs(L - Li) < 1e-6

    nc = _get_nc(L)

    in_maps = []
    for core in range(NCORES):
        s = core // 2
        h = core % 2
        wp = np.roll(wps[s], -1024 * h, axis=0)  # rotated system view
        # replicated across partitions so the load DMA reads distinct
        # addresses at full HBM rate (a stride-0 broadcast read is
        # bank-serialized and ~3x slower).
        wpj = np.ascontiguousarray(
            np.broadcast_to(wp.T.reshape(1, 3 * N), (128, 3 * N))
        )
        wpin = np.empty((128, 3 * NSTRIP), dtype=np.float32)
        for m in range(NSTRIP):
            wpin[:, 3 * m : 3 * m + 3] = -wp[m * 128 : (m + 1) * 128, :]
        in_maps.append({"wpj": wpj, "wpin": wpin})

    res = run_bass_kernel_spmd(nc, in_maps, core_ids=list(range(NCORES)))
    global LAST_RESULT
    LAST_RESULT = res

    # assemble: roll the rotated cores' outputs back and merge (each true
    # block is written by exactly one core; unwritten regions are zero).
    # diff arrives as component planes [3, N, N] -> interleave to [N, N, 3].
    edge_diff = np.zeros((B, N, N, 3), dtype=np.float32)
    edge_dist = np.zeros((B, N, N), dtype=np.float32)
    edge_mask = np.zeros((B, N, N), dtype=np.uint8)
    for core in range(NCORES):
        s = core // 2
        h = core % 2
        out = res.results[core]
        od = out["diff"].reshape(3, N, N)
        osd = out["dist"]
        om = out["mask"]
        if h:
            od = np.roll(od, (1024, 1024), axis=(1, 2))
            osd = np.roll(osd, (1024, 1024), axis=(0, 1))
            om = np.roll(om, (1024, 1024), axis=(0, 1))
        edge_diff[s] += np.moveaxis(od, 0, -1)
        edge_dist[s] += osd
        edge_mask[s] |= om
        # scatter compact mirror blocks: block (m+k, m) in local slots
        dm = out["diffm"].reshape(NSTRIP, KMAX - 1, 128, 3, 128)
        sm = out["distm"].reshape(NSTRIP, KMAX - 1, 128, 128)
        mm = out["maskm"].reshape(NSTRIP, KMAX - 1, 128, 128)
        for m in range(NSTRIP):
            for k in range(1, KMAX):
                trb = ((m + k + 8 * h) % NB) * 128
                tcb = ((m + 8 * h) % NB) * 128
                edge_diff[s, trb : trb + 128, tcb : tcb + 128] = np.moveaxis(
                    dm[m, k - 1], 1, 2
                )
                edge_dist[s, trb : trb + 128, tcb : tcb + 128] = sm[m, k - 1]
                edge_mask[s, trb : trb + 128, tcb : tcb + 128] = mm[m, k - 1]
    return edge_diff, edge_dist, edge_mask.astype(bool)
